# revision 1
# baseline (speedup 1.0000x reference)
"""3-layer GAT on 8 TRN2 NeuronCores.

Strategy (graph/data parallel, per sharding hint):
- Nodes are sharded contiguously across 8 cores by destination id.
- Per layer l, a gather table T_l[n] = [a_dst(n) | h(n) | a_src(n)] is built
  by dense matmul with host-premultiplied weights
  Wext = [W@Ad | W | W@As]  (attention vectors folded into W).
- Edges (with self loops) are grouped by destination block (128 dst nodes),
  sorted by src within a block, padded to K chunks of 128 edges per block.
- Per block: one batched indirect-DMA gather of the K*128 source rows, one
  small indirect gather of the dst a_dst entries, softmax logits
  = leaky_relu(a_src[src]+a_dst[dst]) -> exp (max-subtraction skipped: it
  cancels in alpha), then K matmuls with a one-hot selection matrix S
  (built on DVE via is_equal against an iota row) accumulate
  [sum_e e_e*h_src_e | sum_e e_e] into PSUM. Normalization happens
  post-aggregation (softmax denom is constant within a segment).
- Layer-1 table is computed replicated (x is replicated); layer-2/3 tables
  are computed on the owned shard and AllGathered.
"""

import math

import numpy as np

HEADS = (4, 2, 1)
OUTS = (32, 16, 2)
NEG_SLOPE = 0.2
P = 128

# Full-problem constants (hardcoded; kernel.py must be self-contained).
N_FULL = 100000
E_FULL = 1600000
FIN = 128
N_CORES = 8


# --------------------------------------------------------------------------
# Host-side preprocessing
# --------------------------------------------------------------------------

def _plan_edges(edge_index: np.ndarray, n_nodes: int, n_cores: int):
    """Build per-core gather/scatter index arrays.

    Returns (idx_src, idx_dst, dloc, K, NPC) where arrays have shape
    [n_cores, NB, 128, K]; slot (b, p, j) holds edge p*K+j of block b.
    """
    src = np.concatenate([edge_index[0], np.arange(n_nodes, dtype=np.int64)])
    dst = np.concatenate([edge_index[1], np.arange(n_nodes, dtype=np.int64)])
    src = src.astype(np.int64)
    dst = dst.astype(np.int64)

    n_blocks_tot = math.ceil(n_nodes / P)
    nb = math.ceil(n_blocks_tot / n_cores)  # blocks per core
    npc = nb * P  # nodes per core
    n_pad = n_cores * npc

    blk = dst // P  # global block id, in [0, n_cores*nb)
    # sort by (block, src) - stable ordering gives src-sorted runs per block
    order = np.lexsort((src, blk))
    src_s = src[order]
    dst_s = dst[order]
    blk_s = blk[order]

    counts = np.bincount(blk_s, minlength=n_cores * nb)
    K = int(math.ceil(counts.max() / P))
    cap = K * P

    idx_src = np.zeros((n_cores * nb, cap), dtype=np.int32)
    idx_dst = np.zeros((n_cores * nb, cap), dtype=np.int32)
    dloc = np.full((n_cores * nb, cap), 255.0, dtype=np.float32)

    # slot offsets within each block
    starts = np.zeros(n_cores * nb + 1, dtype=np.int64)
    np.cumsum(counts, out=starts[1:])
    pos_in_blk = np.arange(len(src_s)) - starts[blk_s]
    flat_pos = blk_s * cap + pos_in_blk
    idx_src.reshape(-1)[flat_pos] = src_s.astype(np.int32)
    idx_dst.reshape(-1)[flat_pos] = dst_s.astype(np.int32)
    dloc.reshape(-1)[flat_pos] = (dst_s % P).astype(np.float32)

    # slot i -> (p, j) = (i // K, i % K): reshape [*, 128, K]
    idx_src = idx_src.reshape(n_cores, nb, P, K)
    idx_dst = idx_dst.reshape(n_cores, nb, P, K)
    dloc = dloc.reshape(n_cores, nb, P, K)
    return idx_src, idx_dst, dloc, K, npc, n_pad


def _ext_weights(W: np.ndarray, att_s: np.ndarray, att_d: np.ndarray):
    """Wext = [W@Ad | W | W@As], shape [Fin, 2H + H*Fo]."""
    fin, fall = W.shape
    h, fo = att_s.shape
    As = np.zeros((fall, h), dtype=np.float32)
    Ad = np.zeros((fall, h), dtype=np.float32)
    for hh in range(h):
        As[hh * fo:(hh + 1) * fo, hh] = att_s[hh]
        Ad[hh * fo:(hh + 1) * fo, hh] = att_d[hh]
    return np.concatenate([W @ Ad, W, W @ As], axis=1).astype(np.float32)


# --------------------------------------------------------------------------
# Device program
# --------------------------------------------------------------------------

def build_nc(n_pad: int, npc: int, K: int, n_cores: int, safe_mode: bool = False):
    import concourse.bass as bass
    import concourse.bacc as bacc
    import concourse.mybir as mybir
    import concourse.tile as tile

    dt = mybir.dt
    Alu = mybir.AluOpType
    Act = mybir.ActivationFunctionType

    NB = npc // P
    NCH = n_pad // P  # total node chunks

    # (Fin, H, Fo) per layer
    dims = [(FIN, HEADS[0], OUTS[0]),
            (HEADS[0] * OUTS[0], HEADS[1], OUTS[1]),
            (HEADS[1] * OUTS[1], HEADS[2], OUTS[2])]

    nc = bacc.Bacc()

    x_in = nc.declare_dram_parameter("x", [n_pad, FIN], dt.float32, isOutput=False)
    wexts = []
    for li, (fin, h, fo) in enumerate(dims):
        roww = 2 * h + h * fo
        wexts.append(nc.declare_dram_parameter(
            f"wext{li + 1}", [fin, roww], dt.float32, isOutput=False))
    b1r = nc.declare_dram_parameter("b1r", [P, HEADS[0] * OUTS[0]], dt.float32, isOutput=False)
    b2r = nc.declare_dram_parameter("b2r", [P, HEADS[1] * OUTS[1]], dt.float32, isOutput=False)
    b3r = nc.declare_dram_parameter("b3r", [P, OUTS[2]], dt.float32, isOutput=False)
    brs = [b1r, b2r, b3r]
    iota_in = nc.declare_dram_parameter("iota", [P, P], dt.float32, isOutput=False)
    ident_in = nc.declare_dram_parameter("ident", [P, P], dt.float32, isOutput=False)
    idxs_in = nc.declare_dram_parameter("idxs", [NB, P, K], dt.int32, isOutput=False)
    idxd_in = nc.declare_dram_parameter("idxd", [NB, P, K], dt.int32, isOutput=False)
    idn_in = nc.declare_dram_parameter("idn", [NB, P, 1], dt.int32, isOutput=False)
    dloc_in = nc.declare_dram_parameter("dloc", [NB, P, K], dt.float32, isOutput=False)
    out_p = nc.declare_dram_parameter("out", [npc, OUTS[2]], dt.float32, isOutput=True)

    rg = [list(range(n_cores))]
    ag_space = "Shared" if n_cores > 4 else "Local"

    with tile.TileContext(nc) as tc:
        with (
            tc.tile_pool(name="dram", bufs=1, space="DRAM") as dram,
            tc.tile_pool(name="consts", bufs=1) as cpool,
            tc.tile_pool(name="apool", bufs=3) as apool,
            tc.tile_pool(name="apsum", bufs=1, space="PSUM") as apsum,
            tc.tile_pool(name="bpool", bufs=2) as bpool,
            tc.tile_pool(name="bsmall", bufs=3) as bsmall,
            tc.tile_pool(name="bpsum", bufs=4, space="PSUM") as bpsum,
        ):
            # ---- DRAM internals
            T1 = dram.tile([n_pad, 2 * dims[0][1] + dims[0][1] * dims[0][2]], dt.float32)
            AD1 = dram.tile([n_pad, dims[0][1]], dt.float32)
            x2s = dram.tile([npc, dims[0][1] * dims[0][2]], dt.float32)
            hx2s = dram.tile([npc, 2 * dims[1][1] + dims[1][1] * dims[1][2]], dt.float32)
            ad2s = dram.tile([npc, dims[1][1]], dt.float32)
            T2 = dram.tile([n_pad, 2 * dims[1][1] + dims[1][1] * dims[1][2]],
                           dt.float32, addr_space=ag_space)
            AD2 = dram.tile([n_pad, dims[1][1]], dt.float32, addr_space=ag_space)
            x3s = dram.tile([npc, dims[1][1] * dims[1][2]], dt.float32)
            hx3s = dram.tile([npc, 2 * dims[2][1] + dims[2][1] * dims[2][2]], dt.float32)
            ad3s = dram.tile([npc, dims[2][1]], dt.float32)
            T3 = dram.tile([n_pad, 2 * dims[2][1] + dims[2][1] * dims[2][2]],
                           dt.float32, addr_space=ag_space)
            AD3 = dram.tile([n_pad, dims[2][1]], dt.float32, addr_space=ag_space)

            # ---- constants in SBUF
            ident = cpool.tile([P, P], dt.float32)
            nc.sync.dma_start(ident[:], ident_in[:])
            iota = cpool.tile([P, P], dt.float32)
            nc.sync.dma_start(iota[:], iota_in[:])
            wt = []
            for li, w in enumerate(wexts):
                wtile = cpool.tile(list(w.shape), dt.float32, name=f"wt{li}")
                nc.sync.dma_start(wtile[:], w[:])
                wt.append(wtile)
            bt = []
            for li, b in enumerate(brs):
                btile = cpool.tile(list(b.shape), dt.float32, name=f"bt{li}")
                nc.sync.dma_start(btile[:], b[:])
                bt.append(btile)

            # ---------------- table build ----------------
            def table_phase(src_dram, n_chunks, wtile, dst_dram, fin, ad_dram, h):
                roww = wtile.shape[1]
                for i in range(n_chunks):
                    xi = apool.tile([P, fin], dt.float32, name="xi", tag="xi")
                    nc.sync.dma_start(xi[:], src_dram[i * P:(i + 1) * P, :])
                    xT_ps = apsum.tile([fin, P], dt.float32, name="xT_ps", tag="xT_ps")
                    nc.tensor.transpose(xT_ps[:], xi[:], ident[:])
                    xT = apool.tile([fin, P], dt.float32, name="xT", tag="xT")
                    nc.vector.tensor_copy(xT[:], xT_ps[:])
                    hx_ps = apsum.tile([P, roww], dt.float32, name="hx_ps", tag="hx_ps")
                    nc.tensor.matmul(hx_ps[:], lhsT=xT[:], rhs=wtile[:],
                                     start=True, stop=True)
                    hx = apool.tile([P, roww], dt.float32, name="hx", tag="hx")
                    nc.vector.tensor_copy(hx[:], hx_ps[:])
                    nc.sync.dma_start(dst_dram[i * P:(i + 1) * P, :], hx[:])
                    nc.sync.dma_start(ad_dram[i * P:(i + 1) * P, :], hx[:, 0:h])

            # ---------------- edge phase ----------------
            def edge_phase(li, T, AD, out_dram):
                fin, h, fo = dims[li]
                fall = h * fo
                roww = 2 * h + fall
                rw = fall + h  # matmul rhs width: [msg | e]
                for b in range(NB):
                    ids = bsmall.tile([P, K], dt.int32, name="ids", tag="ids")
                    nc.sync.dma_start(ids[:], idxs_in[b])
                    dl = bsmall.tile([P, K], dt.float32, name="dl", tag="dl")
                    nc.sync.dma_start(dl[:], dloc_in[b])
                    idn = bsmall.tile([P, 1], dt.int32, name="idn", tag="idn")
                    nc.sync.dma_start(idn[:], idn_in[b])
                    # this block's own a_dst rows (one conforming gather)
                    adb = bsmall.tile([P, h], dt.float32, name="adb", tag="adb")
                    nc.gpsimd.indirect_dma_start(
                        out=adb[:], out_offset=None, in_=AD[:],
                        in_offset=bass.IndirectOffsetOnAxis(ap=idn[:], axis=0))

                    # HW indirect DMA supports ONE offset per partition per call
                    # (multi-offset APs misbehave) -> one gather per chunk.
                    G = bpool.tile([P, K * roww], dt.float32, name="G", tag="G")
                    for j in range(K):
                        nc.gpsimd.indirect_dma_start(
                            out=G[:, j * roww:(j + 1) * roww],
                            out_offset=None,
                            in_=T[:],
                            in_offset=bass.IndirectOffsetOnAxis(ap=ids[:, j:j + 1], axis=0),
                        )

                    G3 = G[:].rearrange("p (k r) -> p k r", r=roww)

                    # one-hot selection matrices for all chunks: [p, k, n]
                    S = bpool.tile([P, K * P], dt.float32, name="S", tag="S")
                    nc.vector.tensor_tensor(
                        out=S[:].rearrange("p (k n) -> p k n", n=P),
                        in0=iota[:].unsqueeze(1).broadcast_to([P, K, P]),
                        in1=dl[:].unsqueeze(2).broadcast_to([P, K, P]),
                        op=Alu.is_equal,
                    )

                    # per-edge a_dst via S^T @ adb (PE transpose of S), then
                    # logits = leaky_relu(a_src[src] + a_dst[dst]); e = exp
                    LG = bsmall.tile([P, K * h], dt.float32, name="LG", tag="LG")
                    if safe_mode:
                        idd = bsmall.tile([P, K], dt.int32, name="idd", tag="idd")
                        nc.sync.dma_start(idd[:], idxd_in[b])
                        ADE = bsmall.tile([P, K * h], dt.float32, name="ADE", tag="ADE")
                        for j in range(K):
                            nc.gpsimd.indirect_dma_start(
                                out=ADE[:, j * h:(j + 1) * h],
                                out_offset=None,
                                in_=AD[:],
                                in_offset=bass.IndirectOffsetOnAxis(
                                    ap=idd[:, j:j + 1], axis=0),
                            )
                        nc.vector.tensor_tensor(
                            out=LG[:].rearrange("p (k r) -> p k r", r=h),
                            in0=ADE[:].rearrange("p (k r) -> p k r", r=h),
                            in1=G3[:, :, h + fall:roww],
                            op=Alu.add,
                        )
                    else:
                        for j in range(K):
                            STp = bpsum.tile([P, P], dt.float32, name="STp",
                                             tag="STp", bufs=1)
                            nc.tensor.transpose(STp[:], S[:, j * P:(j + 1) * P], ident[:])
                            ST = bpool.tile([P, P], dt.float32, name="ST", tag="ST")
                            nc.vector.tensor_copy(ST[:], STp[:])
                            adep = bpsum.tile([P, h], dt.float32, name="adep",
                                              tag="adep", bufs=1)
                            nc.tensor.matmul(adep[:], lhsT=ST[:], rhs=adb[:],
                                             start=True, stop=True)
                            nc.vector.tensor_tensor(
                                out=LG[:, j * h:(j + 1) * h],
                                in0=adep[:],
                                in1=G[:, j * roww + h + fall:(j + 1) * roww],
                                op=Alu.add,
                            )
                    TMP = bsmall.tile([P, K * h], dt.float32, name="TMP", tag="TMP")
                    nc.vector.tensor_scalar(
                        out=TMP[:], in0=LG[:], scalar1=NEG_SLOPE, scalar2=None,
                        op0=Alu.mult)
                    nc.vector.tensor_tensor(out=LG[:], in0=LG[:], in1=TMP[:], op=Alu.max)
                    EX = bsmall.tile([P, K * h], dt.float32, name="EX", tag="EX")
                    nc.scalar.activation(EX[:], LG[:], Act.Exp)
                    EX3 = EX[:].rearrange("p (k r) -> p k r", r=h)

                    # rhs = [h_src * e (head-broadcast) | e]
                    R = bpool.tile([P, K * rw], dt.float32, name="R", tag="R")
                    R3 = R[:].rearrange("p (k r) -> p k r", r=rw)
                    nc.vector.tensor_tensor(
                        out=R3[:, :, 0:fall].rearrange("p k (hh f) -> p k hh f", f=fo),
                        in0=G3[:, :, h:h + fall].rearrange("p k (hh f) -> p k hh f", f=fo),
                        in1=EX3.unsqueeze(3).broadcast_to([P, K, h, fo]),
                        op=Alu.mult,
                    )
                    nc.vector.tensor_copy(R3[:, :, fall:rw], EX3)

                    ps = bpsum.tile([P, rw], dt.float32, name="ps", tag="ps")
                    for j in range(K):
                        nc.tensor.matmul(
                            ps[:],
                            lhsT=S[:, j * P:(j + 1) * P],
                            rhs=R[:, j * rw:(j + 1) * rw],
                            start=(j == 0), stop=(j == K - 1),
                        )

                    # normalize by segment softmax denominator
                    den = bsmall.tile([P, h], dt.float32, name="den", tag="den")
                    nc.vector.tensor_scalar(
                        out=den[:], in0=ps[:, fall:rw], scalar1=1e-12, scalar2=None,
                        op0=Alu.add)
                    rec = bsmall.tile([P, h], dt.float32, name="rec", tag="rec")
                    nc.vector.reciprocal(rec[:], den[:])
                    O = bsmall.tile([P, fall], dt.float32, name="O", tag="O")
                    nc.vector.tensor_tensor(
                        out=O[:].rearrange("p (hh f) -> p hh f", f=fo),
                        in0=ps[:, 0:fall].rearrange("p (hh f) -> p hh f", f=fo),
                        in1=rec[:].unsqueeze(2).broadcast_to([P, h, fo]),
                        op=Alu.mult,
                    )
                    # add bias
                    XB = bsmall.tile([P, fall], dt.float32, name="XB", tag="XB")
                    nc.vector.tensor_tensor(out=XB[:], in0=O[:], in1=bt[li][:], op=Alu.add)

                    if li < 2:
                        # ELU = max(x,0) - 1 + exp(min(x,0))
                        TM = bsmall.tile([P, fall], dt.float32, name="TM", tag="TM")
                        nc.vector.tensor_scalar(
                            out=TM[:], in0=XB[:], scalar1=0.0, scalar2=None, op0=Alu.min)
                        TE = bsmall.tile([P, fall], dt.float32, name="TE", tag="TE")
                        nc.scalar.activation(TE[:], TM[:], Act.Exp)
                        TP = bsmall.tile([P, fall], dt.float32, name="TP", tag="TP")
                        nc.vector.tensor_scalar(
                            out=TP[:], in0=XB[:], scalar1=0.0, scalar2=-1.0,
                            op0=Alu.max, op1=Alu.add)
                        XN = bsmall.tile([P, fall], dt.float32, name="XN", tag="XN")
                        nc.vector.tensor_tensor(out=XN[:], in0=TP[:], in1=TE[:], op=Alu.add)
                        nc.sync.dma_start(out_dram[b * P:(b + 1) * P, :], XN[:])
                    else:
                        # log_softmax over the 2 classes
                        MX = bsmall.tile([P, 1], dt.float32, name="MX", tag="MX")
                        nc.vector.reduce_max(MX[:], XB[:], axis=mybir.AxisListType.X)
                        ZC = bsmall.tile([P, fall], dt.float32, name="ZC", tag="ZC")
                        nc.vector.tensor_scalar(
                            out=ZC[:], in0=XB[:], scalar1=MX[:, 0:1], scalar2=None,
                            op0=Alu.subtract)
                        EZ = bsmall.tile([P, fall], dt.float32, name="EZ", tag="EZ")
                        nc.scalar.activation(EZ[:], ZC[:], Act.Exp)
                        SM = bsmall.tile([P, 1], dt.float32, name="SM", tag="SM")
                        nc.vector.reduce_sum(SM[:], EZ[:], axis=mybir.AxisListType.X)
                        LS = bsmall.tile([P, 1], dt.float32, name="LS", tag="LS")
                        nc.scalar.activation(LS[:], SM[:], Act.Ln)
                        FO = bsmall.tile([P, fall], dt.float32, name="FO", tag="FO")
                        nc.vector.tensor_scalar(
                            out=FO[:], in0=ZC[:], scalar1=LS[:, 0:1], scalar2=None,
                            op0=Alu.subtract)
                        nc.sync.dma_start(out_dram[b * P:(b + 1) * P, :], FO[:])

            # ================= Layer 1 =================
            table_phase(x_in, NCH, wt[0], T1, dims[0][0], AD1, dims[0][1])
            edge_phase(0, T1, AD1, x2s)

            # ================= Layer 2 =================
            table_phase(x2s, NB, wt[1], hx2s, dims[1][0], ad2s, dims[1][1])
            nc.gpsimd.collective_compute(
                "AllGather", mybir.AluOpType.bypass, replica_groups=rg,
                ins=[hx2s[:]], outs=[T2[:]])
            nc.gpsimd.collective_compute(
                "AllGather", mybir.AluOpType.bypass, replica_groups=rg,
                ins=[ad2s[:]], outs=[AD2[:]])
            edge_phase(1, T2, AD2, x3s)

            # ================= Layer 3 =================
            table_phase(x3s, NB, wt[2], hx3s, dims[2][0], ad3s, dims[2][1])
            nc.gpsimd.collective_compute(
                "AllGather", mybir.AluOpType.bypass, replica_groups=rg,
                ins=[hx3s[:]], outs=[T3[:]])
            nc.gpsimd.collective_compute(
                "AllGather", mybir.AluOpType.bypass, replica_groups=rg,
                ins=[ad3s[:]], outs=[AD3[:]])
            edge_phase(2, T3, AD3, out_p)

    nc.compile()
    return nc


# --------------------------------------------------------------------------
# Runner
# --------------------------------------------------------------------------

def gat_forward(x, edge_index, W1, att_s1, att_d1, b1, W2, att_s2, att_d2, b2,
                W3, att_s3, att_d3, b3, n_cores=N_CORES, mode="hw", trace=False,
                safe_mode=False):
    x = np.asarray(x, dtype=np.float32)
    n_nodes = x.shape[0]

    idx_src, idx_dst, dloc, K, npc, n_pad = _plan_edges(
        np.asarray(edge_index), n_nodes, n_cores)

    x_pad = np.zeros((n_pad, x.shape[1]), dtype=np.float32)
    x_pad[:n_nodes] = x

    wext1 = _ext_weights(np.asarray(W1, np.float32), np.asarray(att_s1, np.float32),
                         np.asarray(att_d1, np.float32))
    wext2 = _ext_weights(np.asarray(W2, np.float32), np.asarray(att_s2, np.float32),
                         np.asarray(att_d2, np.float32))
    wext3 = _ext_weights(np.asarray(W3, np.float32), np.asarray(att_s3, np.float32),
                         np.asarray(att_d3, np.float32))

    b1r = np.broadcast_to(np.asarray(b1, np.float32), (P, len(b1))).copy()
    b2r = np.broadcast_to(np.asarray(b2, np.float32), (P, len(b2))).copy()
    b3r = np.broadcast_to(np.asarray(b3, np.float32), (P, len(b3))).copy()
    iota = np.broadcast_to(np.arange(P, dtype=np.float32), (P, P)).copy()
    ident = np.eye(P, dtype=np.float32)

    nc = build_nc(n_pad, npc, K, n_cores, safe_mode=safe_mode)

    nb = npc // P
    in_maps = []
    for c in range(n_cores):
        idn = (c * npc + np.arange(npc, dtype=np.int32)).reshape(nb, P, 1)
        in_maps.append({
            "x": x_pad,
            "wext1": wext1, "wext2": wext2, "wext3": wext3,
            "b1r": b1r, "b2r": b2r, "b3r": b3r,
            "iota": iota, "ident": ident,
            "idxs": np.ascontiguousarray(idx_src[c]),
            "idxd": np.ascontiguousarray(idx_dst[c]),
            "idn": idn,
            "dloc": np.ascontiguousarray(dloc[c]),
        })

    if mode == "sim":
        from concourse.bass_interp import MultiCoreSim
        sim = MultiCoreSim(nc, n_cores)
        for c in range(n_cores):
            for k, v in in_maps[c].items():
                sim.cores[c].tensor(k)[:] = v
        sim.simulate()
        outs = [np.array(sim.cores[c].tensor("out")) for c in range(n_cores)]
        res = None
    else:
        from concourse.bass_utils import run_bass_kernel_spmd
        try:
            res = run_bass_kernel_spmd(nc, in_maps, list(range(n_cores)), trace=trace)
        except Exception:
            # the axon-tunneled device can be left wedged by a prior crash;
            # reset once and retry
            try:
                import ctypes
                lib = ctypes.CDLL("/opt/axon/libaxon_pjrt.so")
                lib.axon_reset.restype = ctypes.c_int64
                lib.axon_reset()
            except Exception:
                pass
            res = run_bass_kernel_spmd(nc, in_maps, list(range(n_cores)), trace=trace)
        outs = [res.results[c]["out"] for c in range(n_cores)]

    full = np.concatenate(outs, axis=0)[:n_nodes]
    return full, res


def kernel(x, edge_index, W1, att_s1, att_d1, b1, W2, att_s2, att_d2, b2,
           W3, att_s3, att_d3, b3):
    args = (x, edge_index, W1, att_s1, att_d1, b1,
            W2, att_s2, att_d2, b2, W3, att_s3, att_d3, b3)
    try:
        out, _ = gat_forward(*args, n_cores=N_CORES, mode="hw", trace=False)
    except Exception:
        # fall back to the conservative (all-indirect-gather) variant
        out, _ = gat_forward(*args, n_cores=N_CORES, mode="hw", trace=False,
                             safe_mode=True)
    return out

